# revision 30
# baseline (speedup 1.0000x reference)
"""Trainium2 Bass kernel for the BiDAF-style attention-flow layer.

S[b,t,j] = H.w_h + U.w_u + (H*w_hu).U + bias
c2q      = softmax_j(S) @ U
q2c      = softmax_t(max_j S) @ H   (broadcast over t)
out      = concat([H, c2q, H*c2q, H*q2c], axis=-1)

Sharding: data-parallel over batch B=64 across 8 NeuronCores (8 batches per
core); W/b replicated; no collectives.

Host-side unsharding/layout (no math is done on the host beyond dtype
rounding and transposition):
 - inputs are shipped to the device in bf16, in BOTH the natural [t, d]
   layout and the d-major (transposed) layout the PE needs for the S
   contraction, so no on-chip transposes of H are needed;
 - the device stores only the three derived segments [c2q | H*c2q | H*q2c]
   in bf16; the H echo segment is stitched from the exact f32 input on the
   host. The rel-err budget (2e-2) is ~10x above bf16 rounding (~1e-3).

Device structure (per batch of 8 per core; t-tiles of 128, paired):
 - S computed TRANSPOSED: stp[j, t] (row 64 = sH) via half-batch matmuls
   with batch-constant stationary weights [w_hu*U^T | w_h].
 - ONE exp activation per pair produces both E^T = exp(S_core^T + sU + b)
   and exp(sH) (bias row 64 = 0) into et_ext [65, 256].
 - et_ext transposes back (PE) to [128, 2, 80]; r' = max_j E and Z = sum_j E
   are paired DVE reduces; em = exp(sH) * r'.
 - c2q matmuls fill a paired PSUM tile [128, 2, 256]; seg0 = c2q/Z via
   per-tile scalar activations (scale = 1/Z); seg1 = seg0 * H one paired
   bf16 DVE mult; seg2 = H * q2c on gpsimd/vector; one batched store.
 - Normalization is software-pipelined one pair behind production, and
   emitted at the head of each iteration, with 1/Z computed in the
   production stage, so no engine queue blocks on a cross-engine chain.
"""

import numpy as np
import ml_dtypes

import concourse.bass as bass
import concourse.mybir as mybir
import concourse.tile as tile
from concourse.bass_utils import run_bass_kernel_spmd
from concourse.masks import make_identity

B, T, J, D = 64, 1024, 64, 256
NCORES = 8
BL = B // NCORES  # batches per core
NT = T // 128     # t-tiles per batch
NP = NT // 2      # tile pairs per batch
HSTR = 272        # hb tile stride (cols); keeps blocks 32B-aligned
F32 = mybir.dt.float32
BF16 = mybir.dt.bfloat16
AX = mybir.AxisListType.X
AF = mybir.ActivationFunctionType
MUL = mybir.AluOpType.mult
ADD = mybir.AluOpType.add
MAX = mybir.AluOpType.max


def split_multi_waits(nc, max_waits=1):
    """Walrus in this container rejects instructions with more than a couple
    of embedded sync waits. Hoist extras into standalone EventSemaphore
    instructions right before the offending instruction."""
    n = 0
    for fn in nc.m.functions:
        for bb in fn.blocks:
            new_insts = []
            for inst in bb.instructions:
                si = getattr(inst, "sync_info", None)
                if si is not None and si.on_wait and len(si.on_wait) > max_waits:
                    waits = list(si.on_wait)
                    for w in waits[:-max_waits]:
                        n += 1
                        ev = mybir.InstEventSemaphore(
                            name=f"I-wsplit-{n}", ins=[], outs=[]
                        )
                        ev.engine = inst.engine
                        ev.sync_info = mybir.SyncInfo(on_wait=[w], on_update=[])
                        new_insts.append(ev)
                    inst.sync_info = mybir.SyncInfo(
                        on_wait=waits[-max_waits:], on_update=list(si.on_update)
                    )
                new_insts.append(inst)
            bb.instructions[:] = new_insts
    return n


def build_nc():
    nc = bass.Bass()
    Hb = nc.declare_dram_parameter("Hb", [BL, T, D], BF16, isOutput=False)
    HTU = nc.declare_dram_parameter("HTU", [BL, 2, 128, T + J], BF16,
                                    isOutput=False)
    Ub = nc.declare_dram_parameter("Ub", [BL, J, D], BF16, isOutput=False)
    W = nc.declare_dram_parameter("W", [3 * D], F32, isOutput=False)
    b = nc.declare_dram_parameter("b", [1], F32, isOutput=False)
    out = nc.declare_dram_parameter("out", [BL, T, 3 * D], BF16, isOutput=True)

    with tile.TileContext(nc) as tc:
        with (
            tc.tile_pool(name="singles", bufs=1) as singles,
            tc.tile_pool(name="hpool", bufs=3) as hpool,
            tc.tile_pool(name="htap", bufs=2) as htpool,
            tc.tile_pool(name="upool", bufs=2) as upool,
            tc.tile_pool(name="outp", bufs=3) as outp,
            tc.tile_pool(name="batch", bufs=2) as bpool,
            tc.tile_pool(name="small", bufs=6) as small,
            # PSUM: 2 + 2 + 3 + 1 = 8 banks
            tc.tile_pool(name="ps_s", bufs=1, space="PSUM") as ps_s,
            tc.tile_pool(name="ps_e", bufs=2, space="PSUM") as ps_e,
            tc.tile_pool(name="ps_c", bufs=3, space="PSUM") as ps_c,
            tc.tile_pool(name="ps_q", bufs=1, space="PSUM") as ps_q,
        ):
            ident_bf = singles.tile([128, 128], BF16)
            make_identity(nc, ident_bf[:])
            ones_row_bf = singles.tile([1, 128], BF16)
            nc.vector.memset(ones_row_bf[:], 1.0)

            def load_batch(bi, ldq=None, first=False):
                ldq = ldq or nc.sync
                hb = hpool.tile([128, NT, HSTR], BF16, tag="hb")
                htu = htpool.tile([128, 2, T + J], BF16, tag="hta")
                ub = upool.tile([J, D], BF16, tag="ub")
                nc.vector.memset(hb[:, :, D : D + 1], 1.0)
                if first:
                    nc.sync.dma_start(out=ub[:], in_=Ub[bi])
                    ldq.dma_start(
                        out=htu[:], in_=HTU[bi].rearrange("c p t -> p c t")
                    )
                    nc.sync.dma_start(
                        out=hb[:, :, 0:D],
                        in_=Hb[bi].rearrange("(n p) d -> p n d", p=128),
                    )
                else:
                    nc.sync.dma_start(
                        out=hb[:, :, 0:D],
                        in_=Hb[bi].rearrange("(n p) d -> p n d", p=128),
                    )
                    ldq.dma_start(
                        out=htu[:], in_=HTU[bi].rearrange("c p t -> p c t")
                    )
                    nc.sync.dma_start(out=ub[:], in_=Ub[bi])
                ht = htu[:, :, 0:T]
                ut = htu[:, :, T : T + J]
                return hb, ht, ub, ut

            # w_hu and w_h as [128,2] column blocks (one DMA each)
            whu_col = singles.tile([128, 2], F32)
            wh_col = singles.tile([128, 2], F32)
            nc.sync.dma_start(
                out=whu_col[:, :],
                in_=W[2 * D : 3 * D].rearrange("(k p) -> p k", p=128),
            )
            nc.sync.dma_start(
                out=wh_col[:, :],
                in_=W[0:D].rearrange("(k p) -> p k", p=128),
            )

            def emit_prep(ub, ut):
                # sU + b (row 64 = 0 so the paired exp also yields exp(sH))
                su_scr = bpool.tile([J, D], F32, tag="suscr")
                su_raw = bpool.tile([J, 1], F32, tag="suraw")
                nc.gpsimd.tensor_mul(su_scr[:], ub[:], w_u_bc[:])
                nc.vector.reduce_sum(su_raw[:], su_scr[:], axis=AX, op=ADD)
                su_ext = bpool.tile([J + 1, 1], F32, tag="suext")
                nc.vector.tensor_tensor(
                    out=su_ext[0:J, :], in0=su_raw[:], in1=b_col[:], op=ADD
                )
                nc.vector.memset(su_ext[J : J + 1, :], 0.0)
                # stationary weights [w_hu*U^T | w_h]
                rhs_w = bpool.tile([128, 2, J + 1], BF16, tag="rhsw")
                for c in range(2):
                    nc.scalar.activation(
                        rhs_w[:, c, 0:J], ut[:, c, :], AF.Copy,
                        scale=whu_col[:, c : c + 1],
                    )
                    nc.scalar.copy(rhs_w[:, c, J : J + 1], wh_col[:, c : c + 1])
                return su_ext, rhs_w

            pend_norm = []   # (ot3, hb, cq3, p, zinv), lagged two pairs
            pend_q2c = []    # (q2czt, hb, em, p), lagged two pairs

            def stage_norm(st):
                ot3, hb, cq3, p, zinv, sub = st
                for k in range(2):
                    nc.scalar.activation(
                        ot3[:, 2 * p + k, 0:D], cq3[:, k, :],
                        AF.Copy, scale=zinv[:, 2 * sub + k : 2 * sub + k + 1],
                    )
                eng = nc.vector if p % 2 == 0 else nc.gpsimd
                eng.tensor_tensor(
                    out=ot3[:, 2 * p : 2 * p + 2, D : 2 * D],
                    in0=ot3[:, 2 * p : 2 * p + 2, 0:D],
                    in1=hb[:, 2 * p : 2 * p + 2, 0:D], op=MUL,
                )

            def stage_q2czt(st):
                q2czt, hb, em, p, sub = st
                for k in range(2):
                    ti = 2 * p + k
                    nc.tensor.matmul(
                        q2czt[0:1, :], em[:, 2 * sub + k : 2 * sub + k + 1],
                        hb[:, ti, 0 : D + 1],
                        start=(ti == 0), stop=(ti == NT - 1),
                        skip_group_check=True,
                    )

            cur = load_batch(0, ldq=nc.scalar, first=True)
            # w_u broadcast over 64 partitions (for the sU reduction)
            w_u_bc = singles.tile([J, D], F32)
            wsl = W[D : 2 * D]
            nc.scalar.dma_start(
                out=w_u_bc[:],
                in_=bass.AP(tensor=wsl.tensor, offset=wsl.offset,
                            ap=[[0, J]] + list(wsl.ap)),
            )
            # b broadcast over 64 partitions
            b_col = singles.tile([J, 1], F32)
            bsl = b[0:1]
            nc.scalar.dma_start(
                out=b_col[:],
                in_=bass.AP(tensor=bsl.tensor, offset=bsl.offset,
                            ap=[[0, J]] + list(bsl.ap)),
            )
            pend_tail = None

            for bi in range(BL):
                hb, ht_all, ub, ut = cur
                if bi + 1 < BL:
                    cur = load_batch(bi + 1)
                su_ext, rhs_w = emit_prep(ub, ut)

                # ---- S^T halves ------------------------------------------
                stps = []
                for h in range(2):
                    stp = ps_s.tile([J + 1, T // 2], F32, tag=f"s{h}")
                    for c in range(2):
                        nc.tensor.matmul(
                            stp[:], rhs_w[:, c, :],
                            ht_all[:, c, 512 * h : 512 * (h + 1)],
                            start=(c == 0), stop=(c == 1),
                        )
                    stps.append(stp)

                # previous batch's q2c tail + pass 2 + store, deferred here
                # so the PE never stalls on the q2c round-trip at batch end
                while pend_q2c:
                    stage_q2czt(pend_q2c.pop(0))
                while pend_norm:
                    stage_norm(pend_norm.pop(0))
                if pend_tail is not None:
                    emit_tail(*pend_tail)
                    pend_tail = None

                # ---- paired softmax + c2q pipeline -----------------------
                ot3 = outp.tile([128, NT, 3 * D], BF16, tag="ot")
                q2czt = ps_q.tile([1, D + 1], F32, tag="qz")

                for hf in range(2):
                    stp = stps[hf]
                    # E^T for the whole half (+ exp(sH) in row 64), one op
                    et_ext = small.tile([J + 1, 512], BF16, tag="etx")
                    nc.scalar.activation(et_ext[:], stp[:],
                                         AF.Exp, bias=su_ext[:], scale=1.0)
                    # transpose back, 4 tiles into one PSUM tile
                    etT = ps_e.tile([128, 4, 80], BF16, tag="et")
                    for k in range(4):
                        nc.tensor.transpose(
                            etT[:, k, 0 : J + 1],
                            et_ext[:, 128 * k : 128 * (k + 1)],
                            ident_bf[0 : J + 1, 0 : J + 1],
                        )
                    # softmax stats for all 4 tiles in single DVE ops
                    r = small.tile([128, 4], BF16, tag="r")
                    nc.vector.reduce_max(r[:], etT[:, :, 0:J], axis=AX, op=MAX)
                    zs = small.tile([128, 4], F32, tag="zs")
                    nc.vector.reduce_sum(zs[:], etT[:, :, 0:J], axis=AX, op=ADD)
                    em = small.tile([128, 4], BF16, tag="em")
                    nc.vector.tensor_tensor(
                        out=em[:], in0=etT[:, :, J], in1=r[:], op=MUL
                    )
                    zinv = small.tile([128, 4], F32, tag="zinv")
                    nc.vector.reciprocal(zinv[:], zs[:])
                    for sub in range(2):
                        p = 2 * hf + sub
                        cq3 = ps_c.tile([128, 2, D], F32, tag="cq")
                        for k in range(2):
                            kk = 2 * sub + k
                            nc.tensor.matmul(
                                cq3[:, k, :],
                                et_ext[0:J, 128 * kk : 128 * (kk + 1)],
                                ub[:], start=True, stop=True,
                            )
                        if len(pend_q2c) >= 2:
                            stage_q2czt(pend_q2c.pop(0))
                        if len(pend_norm) >= 2:
                            stage_norm(pend_norm.pop(0))
                        pend_norm.append((ot3, hb, cq3, p, zinv, sub))
                        pend_q2c.append((q2czt, hb, em, p, sub))

                def emit_tail(tbi, thb, tot3, tq2czt):
                    # ---- q2c broadcast + pass 2 + store ------------------
                    ztinv = bpool.tile([1, 1], F32, tag="ztinv")
                    nc.vector.reciprocal(ztinv[:], tq2czt[0:1, D : D + 1])
                    q2c_row = bpool.tile([1, D], BF16, tag="q2crow")
                    nc.vector.tensor_scalar_mul(q2c_row[:], tq2czt[0:1, 0:D],
                                                ztinv[:])
                    q2cbp = ps_q.tile([128, D], F32, tag="qz")
                    nc.tensor.matmul(q2cbp[:], ones_row_bf[:], q2c_row[:],
                                     start=True, stop=True)
                    q2cb = bpool.tile([128, D], BF16, tag="q2cb")
                    nc.scalar.copy(q2cb[:], q2cbp[:])
                    if tbi == BL - 1:
                        # drain: interleave half-stores with seg2s, on both
                        # HWDGE queues, so the final store overlaps compute
                        for hh in range(2):
                            for ti in range(4 * hh, 4 * (hh + 1)):
                                eng = nc.gpsimd if ti % 2 == 0 else nc.vector
                                eng.tensor_mul(
                                    tot3[:, ti, 2 * D : 3 * D],
                                    thb[:, ti, 0:D], q2cb[:],
                                )
                            sl = slice(4 * hh, 4 * (hh + 1))
                            q = nc.sync if hh == 0 else nc.scalar
                            q.dma_start(
                                out=out[tbi].rearrange(
                                    "(n p) c -> p n c", p=128)[:, sl],
                                in_=tot3[:, sl],
                            )
                    else:
                        for ti in range(NT):
                            eng = nc.vector if ti % 4 == 3 else nc.gpsimd
                            eng.tensor_mul(
                                tot3[:, ti, 2 * D : 3 * D], thb[:, ti, 0:D],
                                q2cb[:],
                            )
                        nc.scalar.dma_start(
                            out=out[tbi].rearrange("(n p) c -> p n c", p=128),
                            in_=tot3[:],
                        )

                pend_tail = (bi, hb, ot3, q2czt)

            while pend_q2c:
                stage_q2czt(pend_q2c.pop(0))
            while pend_norm:
                stage_norm(pend_norm.pop(0))
            emit_tail(*pend_tail)

    split_multi_waits(nc)
    return nc


_NC_CACHE = None


def get_nc():
    global _NC_CACHE
    if _NC_CACHE is None:
        _NC_CACHE = build_nc()
    return _NC_CACHE


def make_in_maps(H, U, W, b):
    """Shard + lay out inputs. H/U are shipped bf16 in both the natural and
    the d-major (pre-transposed) layouts; W/b stay f32."""
    H = np.ascontiguousarray(np.asarray(H, dtype=np.float32))
    U = np.ascontiguousarray(np.asarray(U, dtype=np.float32))
    W = np.ascontiguousarray(np.asarray(W, dtype=np.float32))
    b = np.ascontiguousarray(np.asarray(b, dtype=np.float32))
    Hb = H.astype(ml_dtypes.bfloat16)
    Ub = U.astype(ml_dtypes.bfloat16)
    # [B, T(+J), 256] -> [B, 2, 128, T+J]  (chunk, d-within-chunk, t|j)
    HT = Hb.reshape(B, T, 2, 128).transpose(0, 2, 3, 1)
    UT = Ub.reshape(B, J, 2, 128).transpose(0, 2, 3, 1)
    HTU = np.ascontiguousarray(np.concatenate([HT, UT], axis=3))
    return [
        {
            "Hb": Hb[i * BL : (i + 1) * BL],
            "HTU": HTU[i * BL : (i + 1) * BL],
            "Ub": Ub[i * BL : (i + 1) * BL],
            "W": W,
            "b": b,
        }
        for i in range(NCORES)
    ]


def assemble(results, H):
    """Unshard: stitch the exact-f32 H echo segment with the device-computed
    bf16 segments [c2q | H*c2q | H*q2c], upcast to f32."""
    H = np.asarray(H, dtype=np.float32)
    full = np.empty((B, T, 4 * D), dtype=np.float32)
    full[:, :, 0:D] = H
    rest = np.concatenate(
        [np.asarray(results[i]["out"]) for i in range(NCORES)], axis=0
    )
    full[:, :, D:] = rest.astype(np.float32)
    return full


def kernel(H, U, W, b):
    nc = get_nc()
    in_maps = make_in_maps(H, U, W, b)
    res = run_bass_kernel_spmd(nc, in_maps, core_ids=list(range(NCORES)))
    return assemble(res.results, H)
